# revision 27
# baseline (speedup 1.0000x reference)
"""KNN graph kernel (DenseDilatedKnnGraph) for Trainium2, 8 NeuronCores.

Problem: x [2, 192, 8192, 1] fp32 -> edge_index [2, 2, 8192, 9] int32.
reference: L2-normalize x along C, pairwise sq-dists over N, top-9 (k=9,
dilation=1) nearest neighbors (indices), stacked with center indices.

Math: for normalized points, ranking by -dist == ranking by cosine
G = Xn^T Xn. Nearest neighbor is always self (cos=1); device masks the
self column with -20 and finds top-8 of the rest; host prepends self.

Split of work:
  - Host (cheap, O(C*N)): normalize x, cast to fp16, rotate columns per
    core so each core's 2048-query block has its self-diagonal at cols
    [0, 2048). Feed the device fp16 directly (halves input DMA).
  - Device (the O(N^2) part): per 128-query row tile, fp16 Gram (2
    matmul passes per 512-col chunk: channels 0-127, then 128-191
    zero-padded) accumulated in [128, 2048] PSUM quarters. ACT
    evacuates quarters to fp16 g4[8192]; DVE adds -20 eye on the self
    diagonal, tree-folds g4 at the 2x fp16 TT-max rate to V[1024] with
    V[p] = max over comb(p) = { p + 1024*m : m = 0..7 }, then max8 +
    find_index8 on 1024 elems (instead of 2x8192). Top-8 combs provably
    contain the top-8 columns: any comb holding a top-8 element has
    comb-max >= the 8th value >= the comb-max of any comb without one.
    Ships only the 8 comb positions per row (u16).
  - Host: rescores the 8x8=64 candidate columns per row with exact fp64
    dots and takes the true top-8 by (-value, index) == jax top_k order.
"""

import numpy as np

B = 2
C = 192
N = 8192
NCORES = 8
RBLK = N // 4  # 2048 query rows per core
NT = RBLK // 128  # 16 row tiles per core
NEG = -20.0
COMB = 8  # columns per comb; comb(p) = {p + 1024*m}
NV = 1024  # V width (find/max scan size)

_cache = {}


def _build_nc():
    import concourse.bacc as bacc
    import concourse.mybir as mybir
    from concourse.bass import ts
    from concourse.tile import TileContext

    f32 = mybir.dt.float32
    f16 = mybir.dt.float16
    u16 = mybir.dt.uint16

    nc = bacc.Bacc("TRN2")

    xin = nc.dram_tensor("xin", [C, N], f16, kind="ExternalInput")
    idx_out = nc.dram_tensor("idx8", [RBLK, 8], u16, kind="ExternalOutput")

    eye_d = nc.inline_tensor(np.eye(128, dtype=np.float16) * NEG, name="eyeneg")

    DCH = 1024  # input DMA chunk

    with TileContext(nc) as tc:
        with (
            tc.tile_pool(name="consts", bufs=1) as cpool,
            tc.tile_pool(name="xpool", bufs=1) as xpool,
            tc.tile_pool(name="wpool", bufs=2) as wpool,
            tc.tile_pool(name="vpool", bufs=3) as vpool,
            tc.tile_pool(name="gpsum", bufs=2, space="PSUM") as gpsum,
        ):
            eye = cpool.tile([128, 128], f16)
            nc.sync.dma_start(eye, eye_d[:, :])

            # normalized fp16 points straight from the host: channels
            # 0..127 in hA, 128..191 in hBz rows 0..63 (rows 64..127
            # zeroed so a K=128 matmul sees only the 64 B-channels).
            hA = xpool.tile([128, N], f16)
            hBz = xpool.tile([128, N], f16)
            nc.gpsimd.memset(hBz[64:128, :], 0.0)
            # chunk 0 split across queues so the first Gram matmul isn't
            # gated on one serial 256 KB transfer
            for sc in range(4):
                ssl = ts(sc, 256)
                nc.sync.dma_start(hA[:, ssl], xin[0:128, ssl])
                nc.sync.dma_start(hBz[0:64, ssl], xin[128:192, ssl])
            for dc in range(1, N // DCH):
                dsl = ts(dc, DCH)
                nc.sync.dma_start(hA[:, dsl], xin[0:128, dsl])
                nc.sync.dma_start(hBz[0:64, dsl], xin[128:192, dsl])

            # main loop: per row tile, Gram quarters -> fold -> top-8 combs.
            # Quarter i covers cols [2048i, 2048(i+1)). The self-diagonal
            # (cols 128t..128t+127) is always in quarter 0.
            for t in range(NT):
                tsl = ts(t, 128)
                g4 = wpool.tile([128, N], f16, tag="g4")
                for i in range(4):
                    ps = gpsum.tile([128, 2048], f32, tag="ps")
                    for hh in range(4):
                        csl = ts(4 * i + hh, 512)
                        osl = slice(hh * 512, (hh + 1) * 512)
                        nc.tensor.matmul(
                            ps[:, osl], hA[:, tsl], hA[:, csl],
                            start=True, stop=False,
                        )
                        nc.tensor.matmul(
                            ps[:, osl], hBz[:, tsl], hBz[:, csl],
                            start=False, stop=True,
                        )
                    nc.scalar.copy(g4[:, 2048 * i : 2048 * (i + 1)], ps)
                    if i == 0:
                        # knock out the self-match diagonal on the fp16
                        # copy (always within quarter 0); runs while ACT
                        # copies quarters 1-3
                        off = t * 128
                        nc.vector.tensor_add(
                            g4[:, off : off + 128], g4[:, off : off + 128], eye
                        )
                    if i == 2:
                        # half the first fold level only needs quarters
                        # 0 and 2 -> overlap it with quarter 3's copy
                        F1 = vpool.tile([128, 4096], f16, tag="F1")
                        nc.vector.tensor_max(
                            F1[:, 0:2048], g4[:, 0:2048], g4[:, 4096:6144]
                        )
                    if i == 3:
                        nc.vector.tensor_max(
                            F1[:, 2048:4096], g4[:, 2048:4096], g4[:, 6144:8192]
                        )
                F2 = vpool.tile([128, 2048], f16, tag="F2")
                nc.vector.tensor_max(F2, F1[:, 0:2048], F1[:, 2048:4096])
                V = vpool.tile([128, NV], f16, tag="V")
                nc.vector.tensor_max(V, F2[:, 0:NV], F2[:, NV : 2 * NV])
                v8 = vpool.tile([128, 8], f16, tag="v8")
                i8 = vpool.tile([128, 8], u16, tag="i8")
                nc.vector.max(out=v8, in_=V)
                nc.vector.max_index(i8, v8, V)
                nc.sync.dma_start(idx_out[tsl, :], i8)

    nc.compile()
    return nc


def _get_nc():
    if "nc" not in _cache:
        _cache["nc"] = _build_nc()
    return _cache["nc"]


def shard_inputs(x):
    """x: [B, C, N, 1] -> list of 8 per-core input maps: normalized fp16
    points with rotated columns."""
    xs = np.ascontiguousarray(np.asarray(x, dtype=np.float32).reshape(B, C, N))
    rns = 1.0 / np.sqrt((xs * xs).sum(axis=1, keepdims=True))  # [B, 1, N]
    h16 = (xs * rns).astype(np.float16)
    in_maps = []
    for c in range(NCORES):
        b, r = divmod(c, 4)
        s = r * RBLK
        hb = h16[b]
        rot = np.ascontiguousarray(np.roll(hb, -s, axis=1)) if s else hb
        in_maps.append({"xin": rot})
    return in_maps


def assemble(results, x):
    """results: 8 dicts with 'idx8' [RBLK, 8] u16 comb positions.

    comb(p) = {p + 1024*m : m = 0..7} in the core's rotated column space.
    Rescore all 64 candidate columns per row with exact fp64 dots of the
    normalized points and take the true top-8 by (-value, index).
    """
    xs = np.asarray(x, dtype=np.float32).reshape(B, C, N)
    n64 = np.sqrt((xs.astype(np.float64) ** 2).sum(axis=1, keepdims=True))
    xn = np.ascontiguousarray((xs / n64).transpose(0, 2, 1))  # [B, N, C] f64

    nn = np.empty((B, N, 9), np.int32)
    m_off = (np.arange(COMB, dtype=np.int64) * NV)[None, None, :]
    for c in range(NCORES):
        b, r = divmod(c, 4)
        s = r * RBLK
        i8 = results[c]["idx8"].astype(np.int64)  # [RBLK, 8]
        cand = ((i8[:, :, None] + m_off).reshape(RBLK, COMB * 8) + s) % N
        rows = np.arange(s, s + RBLK, dtype=np.int64)
        xnb = xn[b]
        top8 = np.empty((RBLK, 8), np.int64)
        CH = 512
        for r0 in range(0, RBLK, CH):
            cc = cand[r0 : r0 + CH]
            rr = rows[r0 : r0 + CH]
            vals = np.einsum("rkc,rc->rk", xnb[cc], xnb[rr], optimize=True)
            vals[cc == rr[:, None]] = -np.inf
            # guard against duplicate candidate columns (tied-comb edge)
            so = np.argsort(cc, axis=1, kind="stable")
            sc = np.take_along_axis(cc, so, axis=1)
            dup_s = np.zeros_like(sc, dtype=bool)
            dup_s[:, 1:] = sc[:, 1:] == sc[:, :-1]
            dup = np.zeros_like(dup_s)
            np.put_along_axis(dup, so, dup_s, axis=1)
            vals[dup] = -np.inf
            order = np.lexsort((cc, -vals), axis=-1)[:, :8]
            top8[r0 : r0 + CH] = np.take_along_axis(cc, order, axis=1)
        nn[b, s : s + RBLK, 1:9] = top8
        nn[b, s : s + RBLK, 0] = rows
    center = np.broadcast_to(np.arange(N, dtype=np.int32)[None, :, None], (B, N, 9))
    return np.ascontiguousarray(np.stack([nn, center], axis=0).astype(np.int32))


def kernel(x, _trace=False, **trace_kwargs):
    from concourse.bass_utils import run_bass_kernel_spmd

    nc = _get_nc()
    in_maps = shard_inputs(x)
    res = run_bass_kernel_spmd(
        nc, in_maps, core_ids=list(range(NCORES)), trace=_trace, **trace_kwargs
    )
    _cache["last_results"] = res
    return assemble(res.results, x)
